# revision 2
# baseline (speedup 1.0000x reference)
"""Embedding lookup + RMSNorm + tied logits projection on 8 trn2 NeuronCores.

Strategy (vocab-tensor-parallel), v3:
  - RMSNorm is a per-row function of the embedding table, so the host folds it
    into the table: emb_n[i] = emb[i] * rsqrt(mean(emb[i]^2) + eps), bf16.
    final_norm folds into the projection: W_f[v,d] = W[v,d] * fn[d], bf16.
  - Pad vocab 50257 -> 51200 = 8 * 6400. Core c owns vocab rows [c*6400, (c+1)*6400).
  - Phase 1 (replicated on every core): gather h = emb_n[idx] via indirect DMA,
    PE-transpose to hnT [d, t] (bf16), DVE copy PSUM->SBUF.
  - Phase 2: logitsT[v, t] = sum_d WT_f[d, v] * hnT[d, t]; WT_f tile stationary,
    hnT slice moving (512 tokens), f32 PSUM accumulation over 6 k-chunks,
    bf16 output. Tokens processed in pieces (512,512,1024,1024,1024) so the
    first matmuls start after only 4 gathers and later pieces' phase 1
    trickles through earlier pieces' matmul streams.
  - Post-compile pass deletes redundant back-to-back InstLdweights (same AP).
  - Host assembles: concat shard.T over vocab, slice to 50257, reshape [2,2048,V].
"""
import os
import sys

sys.path.insert(0, "/opt/trn_rl_repo")

import numpy as np
import ml_dtypes

import concourse.bass as bass
import concourse.mybir as mybir
import concourse.tile as tile
from concourse import bacc
from concourse.bass import IndirectOffsetOnAxis
from concourse.bass_utils import run_bass_kernel_spmd

f32 = mybir.dt.float32
bf16 = mybir.dt.bfloat16
i32 = mybir.dt.int32

B, S, V, D = 2, 2048, 50257, 768
T = B * S                 # 4096 tokens
NC = 8                    # cores
VS = 6400                 # vocab shard per core (51200 padded)
KK = D // 128             # 6 k-chunks
NTT = T // 128            # 32 token tiles (g-tiles)
PIECES = [4, 4, 8, 8, 8]  # g-tiles per token piece (sum = 32)
NVO = VS // 256           # 25 W-DMA groups
EPS = 1e-5

_cache = {}


def _dedup_ldweights(nc):
    """Drop InstLdweights that reload the exact weights already resident in
    the PE array (same AP, no intervening transpose), when sync-free."""
    removed = 0
    for blk in nc.m.functions[0].blocks:
        cur = None
        keep = []
        for inst in blk.instructions:
            nm = type(inst).__name__
            if nm == "InstLdweights":
                si = inst.sync_info
                clean = si is None or (
                    len(si.on_wait) == 0 and len(si.on_update) == 0
                )
                key = str(inst.ins[0])
                if clean and cur == key:
                    removed += 1
                    continue
                cur = key
            elif nm == "InstMatmult":
                if inst.is_transpose:
                    cur = None
            keep.append(inst)
        if removed:
            blk.instructions[:] = keep
    return removed


def _build():
    nc = bacc.Bacc("TRN2", target_bir_lowering=False, debug=False, num_devices=NC)
    emb = nc.dram_tensor("emb", [V, D], bf16, kind="ExternalInput")
    idx = nc.dram_tensor("idx", [128, NTT], i32, kind="ExternalInput")
    ident_d = nc.dram_tensor("ident", [128, 128], bf16, kind="ExternalInput")
    wt = nc.dram_tensor("wt", [128, KK, VS], bf16, kind="ExternalInput")
    outT = nc.dram_tensor("logitsT", [VS, T], bf16, kind="ExternalOutput")

    piece_g0 = [sum(PIECES[:p]) for p in range(len(PIECES))]  # first g-tile
    piece_t0 = [g * 128 for g in piece_g0]                    # first token

    with tile.TileContext(nc) as tc:
        with (
            tc.tile_pool(name="const", bufs=1) as constp,
            tc.tile_pool(name="hntp", bufs=1) as hntp,
            tc.tile_pool(name="wtp", bufs=3) as wtp,
            tc.tile_pool(name="outp", bufs=6) as outp,
            tc.tile_pool(name="gp", bufs=6) as gp,
            tc.tile_pool(name="tps", bufs=2, space="PSUM") as tps,
            tc.tile_pool(name="mpsum", bufs=6, space="PSUM") as mpp,
        ):
            idxsb = constp.tile([128, NTT], i32)
            nc.sync.dma_start(out=idxsb[:], in_=idx[:])
            ident = constp.tile([128, 128], bf16)
            nc.sync.dma_start(out=ident[:], in_=ident_d[:])
            hnt = [hntp.tile([128, KK, ng * 128], bf16, name=f"hnt{p}")
                   for p, ng in enumerate(PIECES)]

            def gather_tile(g):
                h = gp.tile([128, D], bf16, tag="h", name=f"h_{g}")
                nc.gpsimd.indirect_dma_start(
                    out=h[:], out_offset=None, in_=emb[:],
                    in_offset=IndirectOffsetOnAxis(ap=idxsb[:, g:g + 1], axis=0),
                )
                return h

            def transpose_start(g, h):
                # 6 transposes into one PSUM bank (768 bf16 = 1.5KB fits)
                pt = tps.tile([128, KK, 128], bf16, tag="tp", name=f"pt_{g}")
                for kk in range(KK):
                    nc.tensor.transpose(out=pt[:, kk, :],
                                        in_=h[:, kk * 128:(kk + 1) * 128],
                                        identity=ident[:])
                return pt

            def transpose_flush(p, g, pt):
                off = (g - piece_g0[p]) * 128
                nc.vector.tensor_copy(out=hnt[p][:, :, off:off + 128], in_=pt[:])

            def phase1_tile(p, g):
                h = gather_tile(g)
                pt = transpose_start(g, h)
                transpose_flush(p, g, pt)

            def phase2_piece(p, trickle_p=None):
                tsh = PIECES[p] // 4            # 512-token chunks in piece
                t0 = piece_t0[p]
                pending = []                    # gathered tiles awaiting transpose
                for vo in range(NVO):
                    # feed the next piece's phase 1 between this vo group's
                    # matmuls: the gather runs ahead on DMA, and each PE
                    # transpose slots between matmul groups two vo later, by
                    # which time the gather's data has landed.
                    if trickle_p is not None and vo < PIECES[trickle_p]:
                        tg = piece_g0[trickle_p] + vo
                        pending.append((tg, gather_tile(tg)))
                    th = None
                    if pending and (vo >= 2 or trickle_p is None):
                        tg, th = pending.pop(0)
                        tpt = tps.tile([128, KK, 128], bf16, tag="tp",
                                       name=f"pt_{tg}")
                    wtt = wtp.tile([128, KK, 256], bf16, tag="wt")
                    nc.sync.dma_start(out=wtt[:],
                                      in_=wt[:, :, vo * 256:(vo + 1) * 256])
                    for vi in range(2):
                        v = vo * 2 + vi
                        pss = [mpp.tile([128, 512], f32, tag="mm",
                                        name=f"mm_{p}_{v}_{t}") for t in range(tsh)]
                        for kk in range(KK):
                            for ts in range(tsh):
                                nc.tensor.matmul(
                                    out=pss[ts][:],
                                    lhsT=wtt[:, kk, vi * 128:(vi + 1) * 128],
                                    rhs=hnt[p][:, kk, ts * 512:(ts + 1) * 512],
                                    start=(kk == 0), stop=(kk == KK - 1),
                                )
                            if th is not None and vi == 0:
                                nc.tensor.transpose(
                                    out=tpt[:, kk, :],
                                    in_=th[:, kk * 128:(kk + 1) * 128],
                                    identity=ident[:])
                                if kk == KK - 1:
                                    transpose_flush(trickle_p, tg, tpt)
                        for ts in range(tsh):
                            ob = outp.tile([128, 512], bf16, tag="ob")
                            nc.vector.tensor_copy(out=ob[:], in_=pss[ts][:])
                            nc.sync.dma_start(
                                out=outT[v * 128:(v + 1) * 128,
                                         t0 + ts * 512:t0 + (ts + 1) * 512],
                                in_=ob[:])

            # PE warm-up: dummy matmuls on the first W tile while the idx DMA
            # and first gathers are in flight, so the clock ramp (HAM) is done
            # before the first transposes/matmuls.
            warm = wtp.tile([128, KK, 256], bf16, tag="wt", name="warm")
            nc.sync.dma_start(out=warm[:], in_=wt[:, :, 0:256])
            wps = mpp.tile([128, 512], f32, tag="mm", name="warm_ps")
            for i in range(6):
                nc.tensor.matmul(out=wps[:, 0:256], lhsT=warm[:, 0, 0:128],
                                 rhs=warm[:, 0, :], start=True, stop=True)

            # piece 0's phase 1 runs in the open; every later piece's phase 1
            # trickles through the previous piece's matmul stream.
            for g in range(piece_g0[0], piece_g0[0] + PIECES[0]):
                phase1_tile(0, g)
            for p in range(len(PIECES)):
                phase2_piece(p, trickle_p=p + 1 if p + 1 < len(PIECES) else None)

    nc.compile()
    if os.environ.get("KERNEL_NO_DEDUP", "0") == "1":
        nc._ldw_removed = 0
    else:
        nc._ldw_removed = _dedup_ldweights(nc)
    return nc


def _in_maps(input_sequence, embedding, final_norm, output_embedding):
    idx_np = np.ascontiguousarray(
        np.asarray(input_sequence).astype(np.int32).reshape(NTT, 128).T)
    emb_f = np.asarray(embedding, dtype=np.float32)
    rn = 1.0 / np.sqrt(np.mean(np.square(emb_f), axis=1, keepdims=True) + EPS)
    emb_np = np.ascontiguousarray((emb_f * rn).astype(ml_dtypes.bfloat16))
    ident_np = np.eye(128, dtype=ml_dtypes.bfloat16)
    fn = np.asarray(final_norm, dtype=np.float32)
    w = np.asarray(output_embedding, dtype=np.float32) * fn[None, :]
    w_pad = np.zeros((NC * VS, D), dtype=np.float32)
    w_pad[:V] = w
    maps = []
    for c in range(NC):
        wc = w_pad[c * VS:(c + 1) * VS]                       # [VS, D]
        wtc = np.ascontiguousarray(
            wc.T.reshape(KK, 128, VS).transpose(1, 0, 2)).astype(ml_dtypes.bfloat16)
        maps.append({"emb": emb_np, "idx": idx_np, "ident": ident_np, "wt": wtc})
    return maps


def _run(in_maps, trace=False):
    if "nc" not in _cache:
        _cache["nc"] = _build()
    return run_bass_kernel_spmd(_cache["nc"], in_maps, list(range(NC)), trace=trace)


def kernel(input_sequence, embedding, final_norm, output_embedding):
    maps = _in_maps(input_sequence, embedding, final_norm, output_embedding)
    res = _run(maps)
    full = np.empty((T, NC * VS), dtype=np.float32)
    for c in range(NC):
        full[:, c * VS:(c + 1) * VS] = res.results[c]["logitsT"].T
    return np.ascontiguousarray(full[:, :V]).reshape(B, S, V)

